# revision 1
# baseline (speedup 1.0000x reference)
"""Trainium2 Bass kernel for the CNN_PHMM_VAE loss (pHMM forward algorithm + KLD).

Strategy
--------
Pure data parallel over batch: each of the 8 cores processes 512 batch rows.
Per core, batch rows live on 128 SBUF partitions x 4 groups packed along the
free axis (group stride 66 = 65 motif states + 1 pad column).

The forward recurrence runs in *scaled exponential space*.  State variables are
pre-multiplied by transition factors so the 3-term "prev" combination becomes
pure adds:

    GM[k] = exp(a_M2M[k]) * FM[k]
    GI[k] = exp(a_I2M[k]) * FI[k]
    GD[k] = exp(a_D2M[k]) * FD[k]
    prev[k] = GM[k] + GI[k] + GD[k]           (= exp-space M/I/D -> M input)

Per l-step (all elementwise over [128 part, 264 free], bf16):
    t1   = C1 . GM          C1 = 0.25*exp(a_I2M + a_M2I - a_M2M)
    t2   = C2 . GI          C2 = 0.25*exp(a_I2I)
    s    = GM + GI
    prev = s + GD
    GM'[k+1] = TAB_l[k] . prev[k]     TAB_l[k] = exp(a_M2M[k+1] + e[k, x_l])
    SD1[k+2] = RATIO[k] . GM'[k+1]    RATIO[k] = exp(a_D2M[k+2]+a_M2D[k+1]-a_M2M[k+1])
    GI'  = t1 + t2
    GD'  = scan(ATIL, SD1)            ATIL[t] = exp(a_D2D[t-1]+a_D2M[t]-a_D2M[t-1])
The delete-state chain is a single hardware tensor_tensor_scan along the free
axis; zeros in ATIL/SD1 at each group's first columns reset the recurrence so
four batch groups share one scan.  Every 16 steps the state is rescaled by its
per-row max (log accumulated in f32) to stay in range.

The emission gather e[b, k, x[b,l]] is a pure data reindex done host-side when
building the TAB stream (device DMA-streams one [128,256] bf16 tile per step).

Final:  -log P = -(ln(prev[64]) + c_acc);  KLD reduced on-device; one [1,2]
f32 partial per core, combined on host.
"""

import os
import sys

import numpy as np

if "/opt/trn_rl_repo" not in sys.path:
    sys.path.insert(0, "/opt/trn_rl_repo")

import ml_dtypes

BF16 = np.dtype(ml_dtypes.bfloat16)

# problem constants (hardcoded per the harness contract)
B, K, L, E = 4096, 64, 128, 16
NCORES = 8
BPC = B // NCORES          # 512 batch rows per core
G = BPC // 128             # 4 groups
GS = K + 2                 # group stride 66 (65 states + 1 pad)
F = G * GS                 # 264 free columns of state
RESCALE_EVERY = 16

_CACHE = {}


def _build_program():
    import concourse.bacc as bacc
    import concourse.mybir as mybir
    from concourse.tile import TileContext

    bf16 = mybir.dt.bfloat16
    f32 = mybir.dt.float32
    MULT = mybir.AluOpType.mult
    ADD = mybir.AluOpType.add
    MAX = mybir.AluOpType.max
    X = mybir.AxisListType.X
    ACT = mybir.ActivationFunctionType

    nc = bacc.Bacc("TRN2", target_bir_lowering=False, debug=False,
                   num_devices=NCORES)

    tab_h = nc.declare_dram_parameter("tab", [L, 128, G * K], bf16, isOutput=False)
    c1_h = nc.declare_dram_parameter("c1", [128, F], bf16, isOutput=False)
    c2_h = nc.declare_dram_parameter("c2", [128, F], bf16, isOutput=False)
    atil_h = nc.declare_dram_parameter("atil", [128, F], bf16, isOutput=False)
    ratio_h = nc.declare_dram_parameter("ratio", [128, G * K], bf16, isOutput=False)
    gm0_h = nc.declare_dram_parameter("gm0", [128, F], bf16, isOutput=False)
    gd0_h = nc.declare_dram_parameter("gd0", [128, F], bf16, isOutput=False)
    cinit_h = nc.declare_dram_parameter("cinit", [128, G], f32, isOutput=False)
    mus_h = nc.declare_dram_parameter("mus", [128, G * E], f32, isOutput=False)
    lv_h = nc.declare_dram_parameter("lv", [128, G * E], f32, isOutput=False)
    out_h = nc.declare_dram_parameter("out", [1, 2], f32, isOutput=True)

    with TileContext(nc) as tc:
        with tc.tile_pool(name="consts", bufs=1) as consts, \
             tc.tile_pool(name="state", bufs=1) as state, \
             tc.tile_pool(name="tmps", bufs=2) as tmps, \
             tc.tile_pool(name="stream", bufs=12) as stream, \
             tc.tile_pool(name="psum", bufs=1, space="PSUM") as psum_pool:

            C1 = consts.tile([128, F], bf16)
            nc.sync.dma_start(C1[:], c1_h[:])
            C2 = consts.tile([128, F], bf16)
            nc.sync.dma_start(C2[:], c2_h[:])
            ATIL = consts.tile([128, F], bf16)
            nc.sync.dma_start(ATIL[:], atil_h[:])
            RATIO = consts.tile([128, G * K], bf16)
            nc.sync.dma_start(RATIO[:], ratio_h[:])
            MUS = consts.tile([128, G * E], f32)
            nc.sync.dma_start(MUS[:], mus_h[:])
            LV = consts.tile([128, G * E], f32)
            nc.sync.dma_start(LV[:], lv_h[:])

            GM = state.tile([128, F], bf16)
            nc.sync.dma_start(GM[:], gm0_h[:])
            GD = state.tile([128, F], bf16)
            nc.sync.dma_start(GD[:], gd0_h[:])
            GI = state.tile([128, F], bf16)
            nc.vector.memset(GI[:], 0.0)
            SD1 = state.tile([128, F], bf16)
            nc.vector.memset(SD1[:], 0.0)
            CACC = state.tile([128, G], f32)
            nc.sync.dma_start(CACC[:], cinit_h[:])

            GM3 = GM.rearrange("p (g k) -> p g k", g=G)
            GI3 = GI.rearrange("p (g k) -> p g k", g=G)
            GD3 = GD.rearrange("p (g k) -> p g k", g=G)
            SD13 = SD1.rearrange("p (g k) -> p g k", g=G)
            RATIO3 = RATIO.rearrange("p (g k) -> p g k", g=G)

            def make_prev():
                s = tmps.tile([128, F], bf16, name="s", tag="s")
                nc.vector.tensor_tensor(s[:], GM[:], GI[:], ADD)
                prev = tmps.tile([128, F], bf16, name="prev", tag="prev")
                nc.vector.tensor_tensor(prev[:], s[:], GD[:], ADD)
                return prev

            for l in range(L):
                tab = stream.tile([128, G * K], bf16, name="tab", tag="tab")
                nc.sync.dma_start(tab[:], tab_h[l])
                tab3 = tab.rearrange("p (g k) -> p g k", g=G)

                # insert-state partial products (off critical path -> Pool)
                t1 = tmps.tile([128, F], bf16, name="t1", tag="t1")
                nc.gpsimd.tensor_tensor(t1[:], C1[:], GM[:], MULT)
                t2 = tmps.tile([128, F], bf16, name="t2", tag="t2")
                nc.gpsimd.tensor_tensor(t2[:], C2[:], GI[:], MULT)

                prev = make_prev()
                prev3 = prev.rearrange("p (g k) -> p g k", g=G)

                # GM'[g, 1..64] = tab[g, 0..63] * prev[g, 0..63]
                nc.vector.tensor_tensor(
                    GM3[:, :, 1:K + 1], tab3[:, :, :], prev3[:, :, 0:K], MULT)
                # SD1[g, 2..65] = RATIO[g, 0..63] * GM'[g, 1..64]
                nc.vector.tensor_tensor(
                    SD13[:, :, 2:K + 2], RATIO3[:, :, :], GM3[:, :, 1:K + 1], MULT)
                # GI' = t1 + t2
                nc.vector.tensor_tensor(GI[:], t1[:], t2[:], ADD)
                # GD' = scan(ATIL, SD1) in exp space
                nc.vector.tensor_tensor_scan(GD[:], ATIL[:], SD1[:], 0.0, MULT, ADD)

                if l == 0:
                    # the initial fM[0]=0 mass participates only in step 0;
                    # from step 1 on fM[0] = NEG (zero mass)
                    nc.vector.memset(GM3[:, :, 0], 0.0)

                if l % RESCALE_EVERY == RESCALE_EVERY - 1:
                    i32 = mybir.dt.int32
                    LSR = mybir.AluOpType.logical_shift_right
                    AND = mybir.AluOpType.bitwise_and
                    SUB = mybir.AluOpType.subtract
                    r1 = tmps.tile([128, G], f32, name="r1", tag="r1")
                    nc.vector.tensor_reduce(r1[:], GM3[:, :, 0:K + 1], X, MAX)
                    r2 = tmps.tile([128, G], f32, name="r2", tag="r2")
                    nc.vector.tensor_reduce(r2[:], GD3[:, :, 0:K + 1], X, MAX)
                    r3 = tmps.tile([128, G], f32, name="r3", tag="r3")
                    nc.vector.tensor_reduce(r3[:], GI3[:, :, 0:K + 1], X, MAX)
                    rm = tmps.tile([128, G], f32, name="rm", tag="rm")
                    nc.vector.tensor_tensor(rm[:], r1[:], r2[:], MAX)
                    nc.vector.tensor_tensor(rm[:], rm[:], r3[:], MAX)
                    nc.vector.tensor_scalar_max(rm[:], rm[:], 1e-30)
                    # power-of-two rescale: scale = 2^(e-127) from rm's
                    # exponent bits -> exact state division, no Ln range issue
                    mask = tmps.tile([128, G], i32, name="mask", tag="mask")
                    nc.vector.tensor_scalar(
                        mask[:], rm.bitcast(i32), 0x7F800000, None, AND)
                    rib = tmps.tile([128, G], i32, name="rib", tag="rib")
                    nc.vector.tensor_scalar(
                        rib[:], mask[:], -1, 0x7F000000, MULT, ADD)
                    rinv = tmps.tile([128, G], f32, name="rinv", tag="rinv")
                    nc.vector.tensor_copy(rinv.bitcast(i32), rib[:])
                    es = tmps.tile([128, G], i32, name="es", tag="es")
                    nc.vector.tensor_scalar(es[:], mask[:], 23, None, LSR)
                    ef = tmps.tile([128, G], f32, name="ef", tag="ef")
                    nc.vector.tensor_copy(ef[:], es[:])
                    el = tmps.tile([128, G], f32, name="el", tag="el")
                    nc.vector.tensor_scalar(
                        el[:], ef[:], 127.0, float(np.log(2.0)), SUB, MULT)
                    nc.vector.tensor_tensor(CACC[:], CACC[:], el[:], ADD)
                    for g in range(G):
                        sc = rinv[:, g:g + 1]
                        nc.vector.tensor_scalar_mul(
                            GM3[:, g, 0:K + 1], GM3[:, g, 0:K + 1], sc)
                        nc.vector.tensor_scalar_mul(
                            GI3[:, g, 0:K + 1], GI3[:, g, 0:K + 1], sc)
                        nc.vector.tensor_scalar_mul(
                            GD3[:, g, 0:K + 1], GD3[:, g, 0:K + 1], sc)

            # ---- final readout ----
            prev = make_prev()
            prev3 = prev.rearrange("p (g k) -> p g k", g=G)
            pf = tmps.tile([128, G], f32, name="pf", tag="pf")
            # floor before log so a fully-underflowed row cannot produce -inf
            nc.vector.tensor_scalar_max(pf[:], prev3[:, :, K], 1e-38)
            lnp = tmps.tile([128, G], f32, name="lnp", tag="lnp")
            nc.scalar.activation(lnp[:], pf[:], ACT.Ln)

            BOTH = consts.tile([128, 2 * G], f32)
            nc.vector.tensor_tensor(BOTH[:, 0:G], lnp[:], CACC[:], ADD)

            # KLD pieces: sum_e (logvar - mu^2 - exp(logvar))
            sq = consts.tile([128, G * E], f32)
            nc.scalar.activation(sq[:], MUS[:], ACT.Square)
            elv = consts.tile([128, G * E], f32)
            nc.scalar.activation(elv[:], LV[:], ACT.Exp)
            d1 = consts.tile([128, G * E], f32)
            nc.vector.tensor_sub(d1[:], LV[:], sq[:])
            nc.vector.tensor_sub(d1[:], d1[:], elv[:])
            d13 = d1.rearrange("p (g e) -> p g e", g=G)
            nc.vector.tensor_reduce(BOTH[:, G:2 * G], d13, X, ADD)

            B3 = BOTH.rearrange("p (h g) -> p h g", h=2)
            both2 = consts.tile([128, 2], f32)
            nc.vector.tensor_reduce(both2[:], B3, X, ADD)

            ones = consts.tile([128, 1], f32)
            nc.vector.memset(ones[:], 1.0)
            acc = psum_pool.tile([1, 2], f32)
            nc.tensor.matmul(acc[:], ones[:], both2[:])
            res = consts.tile([1, 2], f32)
            nc.vector.tensor_copy(res[:], acc[:])
            nc.sync.dma_start(out_h[:], res[:])

    nc.compile()
    return nc


def _to_pg(arr):
    """[B, ...] -> [NCORES, 128, G, ...]  with b = c*BPC + g*128 + p."""
    tail = arr.shape[1:]
    return arr.reshape(NCORES, G, 128, *tail).transpose(
        0, 2, 1, *range(3, 3 + len(tail)))


def _pad_state(a65):
    """[B, 65] -> [B, 66] with zero pad column."""
    out = np.zeros((a65.shape[0], GS), a65.dtype)
    out[:, :K + 1] = a65
    return out


def _host_prep(batch_input, transition_probs, emission_probs, mus, logvars):
    x = np.asarray(batch_input, np.int32)
    a = np.asarray(transition_probs, np.float32)
    e = np.asarray(emission_probs, np.float32)
    mus = np.asarray(mus, np.float32)
    lv = np.asarray(logvars, np.float32)

    aM2M, aM2I, aM2D = a[:, :, 0], a[:, :, 1], a[:, :, 2]
    aI2M, aI2I = a[:, :, 3], a[:, :, 4]
    aD2M, aD2D = a[:, :, 5], a[:, :, 6]

    C1 = 0.25 * np.exp(aI2M + aM2I - aM2M)                     # [B,65]
    C2 = 0.25 * np.exp(aI2I)                                   # [B,65]
    ATIL = np.zeros((B, GS), np.float32)
    ATIL[:, 1:K + 1] = np.exp(
        aD2D[:, 0:K] + aD2M[:, 1:K + 1] - aD2M[:, 0:K])
    RATIO = np.zeros((B, K), np.float32)
    RATIO[:, 0:K - 1] = np.exp(
        aD2M[:, 2:K + 1] + aM2D[:, 1:K] - aM2M[:, 1:K])

    # emission tables premultiplied by the next match transition, then
    # gathered by the observed symbols (pure reindex over input data)
    ehat = np.exp(aM2M[:, 1:K + 1, None] + e)                  # [B,K,4]
    TE = ehat[np.arange(B)[:, None, None],
              np.arange(K)[None, :, None],
              x[:, None, :]]                                   # [B,K,L]

    # initial state in log space (single-path delete chain), normalized
    gm0_log = np.full((B, K + 1), -np.inf, np.float32)
    gm0_log[:, 0] = aM2M[:, 0]
    fd0 = np.full((B, K + 1), -np.inf, np.float64)
    fd0[:, 1] = aM2D[:, 0]
    fd0[:, 2:] = aM2D[:, 0:1] + np.cumsum(
        aD2D[:, 1:K].astype(np.float64), axis=1)
    gd0_log = fd0 + aD2M
    gd0_log[:, 0] = -np.inf

    cinit = np.maximum(gm0_log.max(axis=1),
                       gd0_log.max(axis=1).astype(np.float32))  # [B]
    GM0 = np.exp(gm0_log - cinit[:, None]).astype(np.float32)
    GD0 = np.exp(gd0_log - cinit[:, None]).astype(np.float32)

    in_maps = []
    c1_pg = _to_pg(_pad_state(C1)).reshape(NCORES, 128, F).astype(BF16)
    c2_pg = _to_pg(_pad_state(C2)).reshape(NCORES, 128, F).astype(BF16)
    atil_pg = _to_pg(ATIL).reshape(NCORES, 128, F).astype(BF16)
    ratio_pg = _to_pg(RATIO).reshape(NCORES, 128, G * K).astype(BF16)
    gm0_pg = _to_pg(_pad_state(GM0)).reshape(NCORES, 128, F).astype(BF16)
    gd0_pg = _to_pg(_pad_state(GD0)).reshape(NCORES, 128, F).astype(BF16)
    cinit_pg = _to_pg(cinit[:, None]).reshape(NCORES, 128, G).astype(np.float32)
    mus_pg = _to_pg(mus).reshape(NCORES, 128, G * E).astype(np.float32)
    lv_pg = _to_pg(lv).reshape(NCORES, 128, G * E).astype(np.float32)
    # TE [B,K,L] -> [c, L, p, g, K]
    tab_pg = TE.reshape(NCORES, G, 128, K, L).transpose(0, 4, 2, 1, 3) \
        .reshape(NCORES, L, 128, G * K).astype(BF16)

    for c in range(NCORES):
        in_maps.append({
            "tab": np.ascontiguousarray(tab_pg[c]),
            "c1": np.ascontiguousarray(c1_pg[c]),
            "c2": np.ascontiguousarray(c2_pg[c]),
            "atil": np.ascontiguousarray(atil_pg[c]),
            "ratio": np.ascontiguousarray(ratio_pg[c]),
            "gm0": np.ascontiguousarray(gm0_pg[c]),
            "gd0": np.ascontiguousarray(gd0_pg[c]),
            "cinit": np.ascontiguousarray(cinit_pg[c]),
            "mus": np.ascontiguousarray(mus_pg[c]),
            "lv": np.ascontiguousarray(lv_pg[c]),
        })
    return in_maps


def kernel(batch_input, transition_probs, emission_probs, mus, logvars,
           _trace=False, _trace_kwargs=None):
    from concourse.bass_utils import run_bass_kernel_spmd

    if "nc" not in _CACHE:
        _CACHE["nc"] = _build_program()
    nc = _CACHE["nc"]

    in_maps = _host_prep(batch_input, transition_probs, emission_probs,
                         mus, logvars)
    kw = {}
    if _trace:
        kw["trace"] = True
        kw.update(_trace_kwargs or {})
    res = run_bass_kernel_spmd(nc, in_maps, list(range(NCORES)), **kw)
    _CACHE["last_results"] = res

    total = 0.0
    for c in range(NCORES):
        s0, s1 = np.asarray(res.results[c]["out"], np.float64).ravel()
        total += -s0 - 0.5 * s1 - 8.0 * BPC
    return np.float32(total / B)

